# revision 27
# baseline (speedup 1.0000x reference)
# kernel.py — MoE (E=16, top-4) Trainium2 Bass kernel, expert-parallel over 8 cores.
#
# v2 strategy (bf16 expert path, f-half slots):
#   - Router (Linear->ReLU->Linear, top-4 softmax) computed data-parallel in
#     fp32 on each core's 256-token shard; AllGather of the dense combine
#     weights. x^T for the router is pre-transposed on the host (xshT input).
#   - Expert work is decomposed into (expert, f-half, token-range) pieces:
#     each slot carries a 2048-wide f-slice of one expert (W1[:, fr], W2[fr, :])
#     so per-core weight traffic is NSLOT*2048 columns instead of NSLOT full
#     experts.  Partial products over f-halves add up in the output
#     accumulator, so halves combine for free.  Host packs pieces into a
#     uniform per-position capacity profile found by search (min total cap).
#   - All expert-path tensors are bf16 (weights, gathered x, h, y, output
#     accumulator, ReduceScatter); PSUM accumulation stays fp32.  Weights are
#     pre-arranged on the host for large contiguous DMAs.
#   - Dispatch per slot: expert mask -> exclusive cumsum (PE matmuls against
#     triangular constants) -> gate to [lo, lo+cap) -> ONE batched indirect-DMA
#     scatter of token ids -> batched gathers of token rows (bf16).
#   - mm1: h^T = gelu(W1e^T x^T + b1) per 512-col f-group; mm2 with W2 slice
#     SBUF-resident; y scaled by combine weight, scatter-added (CCE add, bf16)
#     into a dense [T,H] accumulator split in two column halves.
#   - ReduceScatter(add, bf16) per half over 8 cores, first half overlapped
#     with second-half compute; each core emits its 256-token fp32 shard.
import numpy as np

H = 1024
F = 4096
FH = 2048                  # f-slice width per slot (half of F)
E = 16
TOPK = 4
T = 2048
NCORES = 8
TSH = T // NCORES          # 256 router tokens per core
DUMP = T                   # dump token row index (row T of the [T+1] buffers)
NEG = -3.0e38
MARGIN = 16                # slack over host-computed counts (host/device drift)
HH = 512                   # output column half width

_CACHE = {}


# ---------------------------------------------------------------------------
# Host-side planning
# ---------------------------------------------------------------------------

def _host_counts(inputs):
    x = np.asarray(inputs["x"], np.float32).reshape(T, H)
    h = np.maximum(x @ np.asarray(inputs["Wr1"], np.float32)
                   + np.asarray(inputs["br1"], np.float32), 0.0)
    lg = h @ np.asarray(inputs["Wr2"], np.float32) + np.asarray(inputs["br2"], np.float32)
    order = np.argsort(-lg, axis=1, kind="stable")[:, :TOPK]
    counts = np.zeros(E, np.int64)
    for e in range(E):
        counts[e] = (order == e).sum()
    return counts


def _try_pack(piece_caps, profile):
    """piece_caps: list of ((e, half), cap) f-half pieces, splittable in token
    ranges.  profile: per-core position caps (each cap has NCORES positions).
    Returns asg[j][core] = ((e, half), lo) or None if infeasible."""
    avail = {}
    for c in profile:
        avail[c] = avail.get(c, 0) + NCORES
    sizes = sorted(set(profile), reverse=True)
    pieces = []
    for key, cap in sorted(piece_caps, key=lambda kc: -kc[1]):
        rem = cap
        lo = 0
        while rem > 0:
            pick = None
            for a in reversed(sizes):        # best fit
                if avail.get(a, 0) > 0 and a >= rem:
                    pick = a
                    break
            if pick is None:
                for a in sizes:              # largest available
                    if avail.get(a, 0) > 0:
                        pick = a
                        break
            if pick is None:
                return None
            avail[pick] -= 1
            pieces.append((pick, key, lo))
            lo += pick
            rem -= pick
    by_cap = {}
    for pc, key, lo in pieces:
        by_cap.setdefault(pc, []).append((key, lo))
    used = {c: 0 for c in set(profile)}
    out = []
    for c in profile:
        pos = []
        for r in range(NCORES):
            i = used[c]
            if i < len(by_cap.get(c, [])):
                pos.append(by_cap[c][i])
            else:
                pos.append(((0, 0), T + 4096))   # empty position
            used[c] += 1
        out.append(pos)
    return out


def _plan(inputs):
    from itertools import combinations_with_replacement

    counts = _host_counts(inputs)
    caps = [int(np.ceil((int(c) + MARGIN) / 128) * 128) for c in counts]
    piece_caps = []
    for e in range(E):
        for hf in range(2):
            piece_caps.append(((e, hf), caps[e]))
    menu = [1024, 896, 768, 640, 512, 384, 256, 128]
    best = None
    for ns in (4, 5, 6):
        for prof in combinations_with_replacement(menu, ns):
            s = sum(prof)
            if not (2 * sum(caps) // NCORES <= s <= 3072):
                continue
            asg = _try_pack(piece_caps, list(prof))
            if asg is not None:
                score = (s, ns)
                if best is None or score < best[0]:
                    best = (score, list(prof), asg)
    if best is None:
        raise RuntimeError(f"no feasible profile for counts {counts}")
    _, prof, asg = best
    return prof, asg


# ---------------------------------------------------------------------------
# Device program
# ---------------------------------------------------------------------------

def _build(profile):
    import concourse.bass as bass
    import concourse.mybir as mybir
    import concourse.tile as tile
    from concourse import bacc
    from concourse.masks import make_identity

    dt = mybir.dt
    f32 = dt.float32
    bf16 = dt.bfloat16
    i32 = dt.int32
    Alu = mybir.AluOpType
    Act = mybir.ActivationFunctionType
    NSLOT = len(profile)
    CMAX = max(profile)
    NFG = FH // 512            # 512-col f-groups per slot (4)
    NFC = FH // 128            # 128-row f-chunks per slot (16)

    nc = bacc.Bacc(None, target_bir_lowering=False, debug=False, num_devices=NCORES)

    # ---------------- I/O ----------------
    xfull16 = nc.dram_tensor("xfull16", [T, H], bf16, kind="ExternalInput")
    xshT = nc.dram_tensor("xshT", [128, H // 128, TSH], f32, kind="ExternalInput")
    Wr1 = nc.dram_tensor("Wr1", [H, H], f32, kind="ExternalInput")
    br1 = nc.dram_tensor("br1", [H], f32, kind="ExternalInput")
    Wr2 = nc.dram_tensor("Wr2", [H, E], f32, kind="ExternalInput")
    br2 = nc.dram_tensor("br2", [E], f32, kind="ExternalInput")
    # weights pre-arranged for big contiguous DMAs (see _in_maps)
    W1L = nc.dram_tensor("W1L", [NSLOT, NFG, 128, 8 * 512], bf16, kind="ExternalInput")
    W2L = nc.dram_tensor("W2L", [NSLOT, 128, NFC, H], bf16, kind="ExternalInput")
    b1L = nc.dram_tensor("b1L", [NSLOT, FH], f32, kind="ExternalInput")
    b2L = nc.dram_tensor("b2L", [NSLOT, H], bf16, kind="ExternalInput")
    ohL = nc.dram_tensor("ohL", [NSLOT, E], f32, kind="ExternalInput")
    slotlo = nc.dram_tensor("slotlo", [NSLOT], f32, kind="ExternalInput")
    out_sh = nc.dram_tensor("out_sh", [TSH, H], f32, kind="ExternalOutput")

    # ---------------- constants ----------------
    u128 = nc.inline_tensor(np.triu(np.ones((128, 128), np.float32), 1), "u128")
    u16 = nc.inline_tensor(np.triu(np.ones((16, 16), np.float32), 1), "u16")
    ones128 = nc.inline_tensor(np.ones((128, 1), np.float32), "ones128")
    tokid_np = (np.arange(16)[None, :] * 128 + np.arange(128)[:, None]).astype(np.int32)
    tokid = nc.inline_tensor(tokid_np, "tokid")
    idxinit = nc.inline_tensor(np.full((CMAX + 1, 1), DUMP, np.int32), "idxinit")

    # ---------------- internal DRAM ----------------
    c2aug = nc.dram_tensor("c2aug", [T + 1, NSLOT], f32)
    # 4 independent scatter targets per slot: splits the token-id scatter's
    # write-after-write chain into 4 parallel chains of 4 ops
    NGRP = 8
    idxbg = [[nc.dram_tensor(f"idxbg{k}_{g}", [profile[k] + 1, 1], i32)
              for g in range(NGRP)] for k in range(NSLOT)]
    outp2 = [nc.dram_tensor(f"outp{h}", [T + 1, HH], bf16) for h in range(2)]
    agin = nc.dram_tensor("agin", [TSH, E], f32)
    call = nc.dram_tensor("call", [T, E], f32, addr_space="Shared")
    rsout2 = [nc.dram_tensor(f"rsout{h}", [TSH, HH], bf16) for h in range(2)]

    RG = [list(range(NCORES))]

    with tile.TileContext(nc, pool_alloc_mode="queue") as tc:
        with (
            tc.tile_pool(name="const", bufs=1) as constp,
            tc.tile_pool(name="persist", bufs=1) as persist,
        ):
            ident16 = constp.tile([128, 128], bf16)
            make_identity(nc, ident16)
            u128_sb = constp.tile_from(u128.ap())
            u16_sb = constp.tile_from(u16.ap())
            ones128_sb = constp.tile_from(ones128.ap())
            tokid_sb = constp.tile_from(tokid.ap())
            onesmm_f32 = constp.tile([1, 128], f32)
            nc.vector.memset(onesmm_f32[:], 1.0)
            onesmm_sb = constp.tile([1, 128], bf16)
            nc.vector.tensor_copy(onesmm_sb[:], onesmm_f32[:])
            zero_sb = constp.tile([128, HH], bf16)
            nc.vector.memset(zero_sb[:], 0.0)
            zero_f32 = constp.tile([1, NSLOT], f32)
            nc.vector.memset(zero_f32[:], 0.0)

            # ====== phase 1: router on this core's 256-token shard (fp32) ======
            with (
                tc.tile_pool(name="rweights", bufs=1) as rw,
                tc.tile_pool(name="rtmp", bufs=3) as rt,
                tc.tile_pool(name="rpsum", bufs=2, space="PSUM") as rp,
            ):
                xt_sh = rw.tile([128, H // 128, TSH], f32)
                nc.sync.dma_start(xt_sh[:], xshT.ap())
                # stream Wr1 per 128-col output block so mm1 starts early
                # (small per-ho tiles keep the router pool footprint low)
                wr1_t = []
                for ho in range(H // 128):
                    t = rw.tile([128, H // 128, 128], f32, tag=f"wr1_{ho}",
                                name=f"wr1_{ho}")
                    nc.sync.dma_start(
                        t[:],
                        Wr1.ap()[:, ho * 128:(ho + 1) * 128]
                        .rearrange("(c p) o -> p c o", p=128))
                    wr1_t.append(t)
                wr2_sb = rw.tile([128, H // 128, E], f32)
                nc.sync.dma_start(wr2_sb[:], Wr2.ap().rearrange("(c p) e -> p c e", p=128))
                br1_sb = rw.tile([128, H // 128], f32)
                nc.sync.dma_start(br1_sb[:], br1.ap().rearrange("(c p) -> p c", p=128))
                br2_rep = rw.tile([128, E], f32)
                nc.sync.dma_start(
                    br2_rep[:],
                    br2.ap().rearrange("(o e) -> o e", o=1).to_broadcast([128, E]))

                r1t = rw.tile([128, H // 128, TSH], f32)
                for ho in range(H // 128):
                    p1 = rp.tile([128, TSH], f32, tag="p1")
                    for hc in range(H // 128):
                        nc.tensor.matmul(
                            p1[:], wr1_t[ho][:, hc, :], xt_sh[:, hc, :],
                            start=(hc == 0), stop=(hc == H // 128 - 1))
                    nc.scalar.activation(r1t[:, ho, :], p1[:], Act.Relu,
                                         bias=br1_sb[:, ho:ho + 1])

                for t2 in range(TSH // 128):
                    p2 = rp.tile([128, E], f32, tag="p2")
                    for hc in range(H // 128):
                        nc.tensor.matmul(
                            p2[:], r1t[:, hc, t2 * 128:(t2 + 1) * 128], wr2_sb[:, hc, :],
                            start=(hc == 0), stop=(hc == H // 128 - 1))
                    lg = rt.tile([128, E], f32, tag="lg")
                    nc.vector.tensor_tensor(lg[:], p2[:], br2_rep[:], op=Alu.add)
                    mx8 = rt.tile([128, 8], f32, tag="mx8")
                    nc.vector.max(mx8[:], lg[:])
                    mx4 = rt.tile([128, 8], f32, tag="mx4")
                    nc.vector.memset(mx4[:], NEG)
                    nc.vector.tensor_copy(mx4[:, 0:TOPK], mx8[:, 0:TOPK])
                    zap = rt.tile([128, E], f32, tag="zap")
                    nc.vector.match_replace(zap[:], in_to_replace=mx4[:], in_values=lg[:],
                                            imm_value=NEG)
                    mask = rt.tile([128, E], f32, tag="mask")
                    nc.vector.tensor_tensor(mask[:], lg[:], zap[:], op=Alu.not_equal)
                    negmax = rt.tile([128, 1], f32, tag="negmax")
                    nc.vector.tensor_scalar_mul(negmax[:], mx8[:, 0:1], -1.0)
                    ex = rt.tile([128, E], f32, tag="ex")
                    nc.scalar.activation(ex[:], lg[:], Act.Exp, bias=negmax[:])
                    nc.vector.tensor_tensor(ex[:], ex[:], mask[:], op=Alu.mult)
                    den = rt.tile([128, 1], f32, tag="den")
                    nc.vector.reduce_sum(den[:], ex[:], axis=mybir.AxisListType.X)
                    rcp = rt.tile([128, 1], f32, tag="rcp")
                    nc.vector.reciprocal(rcp[:], den[:])
                    csh = rt.tile([128, E], f32, tag="csh")
                    nc.vector.tensor_scalar(csh[:], ex[:], rcp[:], None, op0=Alu.mult)
                    nc.sync.dma_start(agin[t2 * 128:(t2 + 1) * 128, :], csh[:])

            ohrep = persist.tile([128, NSLOT, E], f32, tag="ohrep")
            nc.sync.dma_start(
                ohrep[:],
                ohL.ap().rearrange("(o l) e -> o l e", o=1).to_broadcast([128, NSLOT, E]))
            lo_rep = persist.tile([128, NSLOT], f32, tag="lo_rep")
            nc.sync.dma_start(
                lo_rep[:],
                slotlo.ap().rearrange("(o l) -> o l", o=1).to_broadcast([128, NSLOT]))

            nc.gpsimd.collective_compute(
                "AllGather", Alu.bypass, replica_groups=RG,
                ins=[agin.ap().opt()], outs=[call.ap().opt()])

            # deferred init (Activation queue is idle while the router computes)
            for k in range(NSLOT):
                for g in range(NGRP):
                    nc.scalar.dma_start(idxbg[k][g][:],
                                        idxinit.ap()[0:profile[k] + 1, :])
            nc.sync.dma_start(c2aug[T:T + 1, :], zero_f32[:])
            for h in range(2):
                for k in range(T // 128):
                    nc.sync.dma_start(outp2[h][k * 128:(k + 1) * 128, :], zero_sb[:])
                nc.sync.dma_start(outp2[h][T:T + 1, :], zero_sb[0:1, :])

            # ====== phase 2: dispatch for the NSLOT local slots ======
            idx_sb = []
            s_col = []
            with (
                tc.tile_pool(name="dsb", bufs=3) as dsb,
                tc.tile_pool(name="dps", bufs=2, space="PSUM") as dps,
            ):
                cf = persist.tile([128, T // 128, E], f32, tag="cfall")
                nc.sync.dma_start(cf[:], call.ap().rearrange("(c p) e -> p c e", p=128))
                c2sb = persist.tile([128, T // 128, NSLOT], f32)
                xg_hold = [persist.tile([128, profile[k] // 128, H], bf16,
                                        tag=f"xgh{k}", name=f"xgh{k}")
                           for k in range(NSLOT)]
                for k in range(NSLOT):
                    idx_sb.append(persist.tile([128, profile[k] // 128], i32,
                                               tag=f"idx{k}", name=f"idx{k}"))
                    s_col.append(persist.tile([128, profile[k] // 128], f32,
                                              tag=f"scol{k}", name=f"scol{k}"))

                # pre-pass: per-slot combine weights ce -> c2sb, then one
                # c2aug store (needed by all scol gathers)
                for k in range(NSLOT):
                    msk = dsb.tile([128, T // 128, E], f32, tag="msk")
                    nc.vector.tensor_tensor(
                        msk[:], cf[:],
                        ohrep[:, k:k + 1, :].to_broadcast([128, T // 128, E]),
                        op=Alu.mult)
                    ce = dsb.tile([128, T // 128], f32, tag="ce")
                    nc.vector.reduce_sum(ce[:], msk[:], axis=mybir.AxisListType.X)
                    nc.vector.tensor_copy(c2sb[:, :, k], ce[:])
                nc.sync.dma_start(
                    c2aug.ap()[0:T, :].rearrange("(c p) l -> p c l", p=128),
                    c2sb[:])

                for k in range(NSLOT):
                    A = profile[k]
                    m = dsb.tile([128, T // 128], f32, tag="m")
                    nc.vector.tensor_scalar(m[:], c2sb[:, :, k], 0.0, None,
                                            op0=Alu.not_equal)

                    # exclusive cumsum over global token order
                    csp = dps.tile([16, 1], f32, tag="csp")
                    nc.tensor.matmul(csp[:], m[:], ones128_sb[:], start=True, stop=True)
                    cs_sb = dsb.tile([16, 1], f32, tag="cs_sb")
                    nc.any.tensor_copy(cs_sb[:], csp[:])
                    csrep = dsb.tile([16, 128], f32, tag="csrep")
                    nc.vector.tensor_copy(csrep[:], cs_sb[:].to_broadcast([16, 128]))
                    posp = dps.tile([128, T // 128], f32, tag="posp")
                    nc.tensor.matmul(posp[:], u128_sb[:], m[:], start=True, stop=False)
                    nc.tensor.matmul(posp[:], csrep[:], u16_sb[:], start=False, stop=True)

                    # gate to [lo, lo+A)
                    tpos = dsb.tile([128, T // 128], f32, tag="tpos")
                    nc.vector.tensor_scalar(tpos[:], posp[:], lo_rep[:, k:k + 1], None,
                                            op0=Alu.subtract)
                    g1 = dsb.tile([128, T // 128], f32, tag="g1")
                    nc.vector.tensor_scalar(g1[:], tpos[:], 0.0, None, op0=Alu.is_ge)
                    g2 = dsb.tile([128, T // 128], f32, tag="g2")
                    nc.vector.tensor_scalar(g2[:], tpos[:], float(A), None, op0=Alu.is_lt)
                    nc.vector.tensor_tensor(m[:], m[:], g1[:], op=Alu.mult)
                    nc.vector.tensor_tensor(m[:], m[:], g2[:], op=Alu.mult)

                    # offsets: O = A + m*(tpos - A)   (unselected -> dump slot A)
                    of = dsb.tile([128, T // 128], f32, tag="of")
                    nc.vector.tensor_scalar(of[:], tpos[:], float(A), None, op0=Alu.subtract)
                    nc.vector.tensor_tensor(of[:], of[:], m[:], op=Alu.mult)
                    nc.vector.tensor_scalar(of[:], of[:], float(A), None, op0=Alu.add)
                    oi = dsb.tile([128, T // 128], i32, tag="oi")
                    nc.vector.tensor_copy(oi[:], of[:])

                    # scatter token ids (single-column [128,1] offset APs; 4
                    # independent target tensors so the WAW chains run in
                    # parallel)
                    for g in range(NGRP):
                        for c in range(T // 128 // NGRP):
                            cc = g * (T // 128 // NGRP) + c
                            nc.gpsimd.indirect_dma_start(
                                out=idxbg[k][g].ap(),
                                out_offset=bass.IndirectOffsetOnAxis(
                                    ap=oi[:, cc:cc + 1], axis=0),
                                in_=tokid_sb[:, cc:cc + 1], in_offset=None,
                                bounds_check=A, oob_is_err=False)

                    # merge the 4 scatter groups (min; DUMP is the identity)
                    # and this slot's gathers, all slot-local and all on the
                    # Pool/DVE queues so later slots never block earlier ones
                    NCK = A // 128
                    tg = []
                    for g in range(NGRP):
                        t = dsb.tile([128, CMAX // 128], i32, tag=f"tg{g}")
                        nc.gpsimd.dma_start(
                            t[:, 0:NCK],
                            idxbg[k][g].ap()[0:A, :].rearrange("(c p) o -> p (c o)", p=128))
                        tg.append(t)
                    # tree min-reduce of the NGRP scatter groups
                    stride = 1
                    while stride < NGRP:
                        for g in range(0, NGRP, 2 * stride):
                            nc.vector.tensor_tensor(
                                tg[g][:, 0:NCK], tg[g][:, 0:NCK],
                                tg[g + stride][:, 0:NCK], op=Alu.min)
                        stride *= 2
                    nc.vector.tensor_copy(idx_sb[k][:], tg[0][:, 0:NCK])
                    for ck in range(NCK):
                        nc.gpsimd.indirect_dma_start(
                            out=xg_hold[k][:, ck, :], out_offset=None,
                            in_=xfull16.ap(),
                            in_offset=bass.IndirectOffsetOnAxis(
                                ap=idx_sb[k][:, ck:ck + 1], axis=0),
                            bounds_check=T - 1, oob_is_err=False)
                    for ck in range(NCK):
                        nc.gpsimd.indirect_dma_start(
                            out=s_col[k][:, ck:ck + 1], out_offset=None,
                            in_=c2aug.ap(),
                            in_offset=bass.IndirectOffsetOnAxis(
                                ap=idx_sb[k][:, ck:ck + 1], axis=0),
                            element_offset=k,
                            bounds_check=T, oob_is_err=True)

            # ====== phase 3: expert MLP per slot ======
            with (
                tc.tile_pool(name="w1", bufs=2) as w1p,
                tc.tile_pool(name="w2", bufs=1) as w2p,
                tc.tile_pool(name="hbuf", bufs=1) as hbp,
                tc.tile_pool(name="xt", bufs=2) as xtp,
                tc.tile_pool(name="ysb", bufs=2) as ysp,
                tc.tile_pool(name="bias", bufs=1) as biasp,
                tc.tile_pool(name="psh", bufs=3, space="PSUM") as psh,
                tc.tile_pool(name="psy", bufs=4, space="PSUM") as psy,
            ):
                b1_sb = biasp.tile([128, NSLOT, NFC], f32)
                nc.sync.dma_start(b1_sb[:], b1L.ap().rearrange("l (c p) -> p l c", p=128))

                for k in range(NSLOT):
                    A = profile[k]
                    NCK = A // 128
                    # transpose gathered x rows via the DMA xbar (keeps PE free);
                    # emitted FIRST so the SP queue feeds next slot's mm1 before
                    # loading its W2
                    xt = xtp.tile([128, H // 128, CMAX], bf16, tag="xt")
                    for ck in range(NCK):
                        nc.sync.dma_start_transpose(
                            xt[:, :, ck * 128:(ck + 1) * 128],
                            xg_hold[k][:, ck, :])

                    b2_sb = biasp.tile([1, H], bf16, tag="b2_sb")
                    nc.sync.dma_start(b2_sb[:], b2L.ap()[k:k + 1, :])

                    # mm1: h^T[f, c] = gelu(sum_h W1[h,f]^T x^T[h,c] + b1[f])
                    chs = [A] if A <= 512 else ([A // 2, A // 2] if A <= 1024
                                                else [512, 512, A - 1024])
                    hbuf = hbp.tile([128, NFC, CMAX], bf16, tag="hbuf")
                    for fo in range(NFG):
                        w1f = w1p.tile([128, 8, 512], bf16, tag="w1f")
                        nc.sync.dma_start(
                            w1f[:], W1L[k, fo].rearrange("p (c f) -> p c f", c=8))
                        for fi in range(4):
                            fg = fo * 4 + fi
                            cc0 = 0
                            for ch in chs:
                                ph = psh.tile([128, 512], f32, tag="ph")
                                for hc in range(H // 128):
                                    nc.tensor.matmul(
                                        ph[:, 0:ch],
                                        w1f[:, hc, fi * 128:(fi + 1) * 128],
                                        xt[:, hc, cc0:cc0 + ch],
                                        start=(hc == 0), stop=(hc == H // 128 - 1))
                                nc.scalar.activation(
                                    hbuf[:, fg, cc0:cc0 + ch], ph[:, 0:ch],
                                    Act.Gelu, bias=b1_sb[:, k, fg:fg + 1])
                                cc0 += ch

                    # W2 slice resident for the slot; the DMA overlaps the
                    # mm1 tail (emitted after mm1's w1f loads on the SP queue)
                    w2r = w2p.tile([128, NFC, H], bf16, tag="w2r")
                    nc.sync.dma_start(w2r[:], W2L[k])

                    # mm2: y[c, h] = (sum_f h^T[f,c]^T W2[f,h] + b2[h]) * s[c]
                    for hh in range(2):
                        for tb0 in range(0, NCK, 4):
                            tbn = min(4, NCK - tb0)
                            pys = [psy.tile([128, HH], f32, tag="py", name=f"py{_i}")
                                   for _i in range(tbn)]
                            for fc in range(NFC):
                                for i in range(tbn):
                                    ck = tb0 + i
                                    nc.tensor.matmul(
                                        pys[i][:],
                                        hbuf[:, fc, ck * 128:(ck + 1) * 128],
                                        w2r[:, fc, hh * HH:(hh + 1) * HH],
                                        start=(fc == 0), stop=False)
                            ysb = ysp.tile([128, 4, HH], bf16, tag="ysb")
                            for i in range(tbn):
                                ck = tb0 + i
                                nc.tensor.matmul(
                                    pys[i][:], onesmm_sb[0:1, :],
                                    b2_sb[0:1, hh * HH:(hh + 1) * HH],
                                    start=False, stop=True)
                                nc.scalar.activation(
                                    ysb[:, i, :], pys[i][:], Act.Copy,
                                    scale=s_col[k][:, ck:ck + 1])
                                nc.gpsimd.indirect_dma_start(
                                    out=outp2[hh].ap(),
                                    out_offset=bass.IndirectOffsetOnAxis(
                                        ap=idx_sb[k][:, ck:ck + 1], axis=0),
                                    in_=ysb[:, i, :], in_offset=None,
                                    compute_op=Alu.add,
                                    bounds_check=T, oob_is_err=True)
                        if k == NSLOT - 1 and hh == 0:
                            # all h-half-0 contributions are in: overlap its
                            # ReduceScatter with h-half-1 compute
                            nc.gpsimd.collective_compute(
                                "ReduceScatter", Alu.add, replica_groups=RG,
                                ins=[outp2[0].ap()[0:T, :].opt()],
                                outs=[rsout2[0].ap().opt()])

            # ====== phase 4: remaining reduce + output shard ======
            with tc.tile_pool(name="outc", bufs=2) as outc:
                for k in range(TSH // 128):
                    ot = outc.tile([128, HH], bf16, tag="ot")
                    nc.sync.dma_start(ot[:], rsout2[0][k * 128:(k + 1) * 128, :])
                    otf = outc.tile([128, HH], f32, tag="otf")
                    nc.vector.tensor_copy(otf[:], ot[:])
                    nc.sync.dma_start(out_sh[k * 128:(k + 1) * 128, 0:HH], otf[:])
                nc.gpsimd.collective_compute(
                    "ReduceScatter", Alu.add, replica_groups=RG,
                    ins=[outp2[1].ap()[0:T, :].opt()], outs=[rsout2[1].ap().opt()])
                for k in range(TSH // 128):
                    ot = outc.tile([128, HH], bf16, tag="ot")
                    nc.sync.dma_start(ot[:], rsout2[1][k * 128:(k + 1) * 128, :])
                    otf = outc.tile([128, HH], f32, tag="otf")
                    nc.vector.tensor_copy(otf[:], ot[:])
                    nc.sync.dma_start(out_sh[k * 128:(k + 1) * 128, HH:H], otf[:])

    nc.compile()
    if not nc.is_finalized():
        nc.finalize()
    return nc


# ---------------------------------------------------------------------------
# Host-side input preparation
# ---------------------------------------------------------------------------

def _in_maps(inputs, profile, asg):
    import ml_dtypes
    bf16 = ml_dtypes.bfloat16

    NSLOT = len(profile)
    NFG = FH // 512
    NFC = FH // 128
    x = np.ascontiguousarray(np.asarray(inputs["x"], np.float32).reshape(T, H))
    W1 = np.asarray(inputs["W1"], np.float32)
    b1 = np.asarray(inputs["b1"], np.float32)
    W2 = np.asarray(inputs["W2"], np.float32)
    b2 = np.asarray(inputs["b2"], np.float32)
    common = {
        "xfull16": np.ascontiguousarray(x.astype(bf16)),
        "Wr1": np.ascontiguousarray(np.asarray(inputs["Wr1"], np.float32)),
        "br1": np.ascontiguousarray(np.asarray(inputs["br1"], np.float32)),
        "Wr2": np.ascontiguousarray(np.asarray(inputs["Wr2"], np.float32)),
        "br2": np.ascontiguousarray(np.asarray(inputs["br2"], np.float32)),
    }
    maps = []
    for r in range(NCORES):
        w1l = np.empty((NSLOT, NFG, 128, 8 * 512), bf16)
        w2l = np.empty((NSLOT, 128, NFC, H), bf16)
        b1l = np.zeros((NSLOT, FH), np.float32)
        b2l = np.zeros((NSLOT, H), bf16)
        oh = np.zeros((NSLOT, E), np.float32)
        lo = np.zeros((NSLOT,), np.float32)
        for k in range(NSLOT):
            (e, hf), l0 = asg[k][r]
            f0 = hf * FH
            w1h = W1[e][:, f0:f0 + FH]                    # [H, FH]
            # W1L[k, fo, p, hc*512+fc] = w1h[hc*128+p, fo*512+fc]
            w1l[k] = (w1h.reshape(8, 128, NFG, 512)       # hc, p, fo, fc
                      .transpose(2, 1, 0, 3)              # fo, p, hc, fc
                      .reshape(NFG, 128, 8 * 512).astype(bf16))
            w2h = W2[e][f0:f0 + FH, :]                    # [FH, H]
            # W2L[k, p, fc, ho] = w2h[fc*128+p, ho]
            w2l[k] = (w2h.reshape(NFC, 128, H)
                      .transpose(1, 0, 2).astype(bf16))
            b1l[k] = b1[e][f0:f0 + FH]
            if hf == 0:
                b2l[k] = b2[e].astype(bf16)
            if l0 <= T:
                oh[k, e] = 1.0       # empty slots keep an all-zero one-hot
            lo[k] = float(l0)
        # shard x^T for the router: xshT[p, hc, t] = x[r*TSH+t, hc*128+p]
        xs = x[r * TSH:(r + 1) * TSH]                     # [TSH, H]
        xshT = np.ascontiguousarray(
            xs.T.reshape(8, 128, TSH).transpose(1, 0, 2))
        maps.append({
            **common,
            "xshT": xshT,
            "W1L": w1l, "W2L": w2l, "b1L": b1l, "b2L": b2l,
            "ohL": oh, "slotlo": lo,
        })
    return maps


def _get_nc(profile):
    key = tuple(profile)
    if key not in _CACHE:
        _CACHE[key] = _build(list(key))
    return _CACHE[key]


def kernel(**inputs) -> np.ndarray:
    from concourse.bass_utils import run_bass_kernel_spmd

    profile, asg = _plan(inputs)
    nc = _get_nc(profile)
    maps = _in_maps(inputs, profile, asg)
    res = run_bass_kernel_spmd(nc, maps, core_ids=list(range(NCORES)))
    shards = [res.results[r]["out_sh"] for r in range(NCORES)]
    out = np.concatenate(shards, axis=0).reshape(np.asarray(inputs["x"]).shape)
    return out.astype(np.float32)


# revision 28
# speedup vs baseline: 1.5485x; 1.5485x over previous
# kernel.py — MoE (E=16, top-4) Trainium2 Bass kernel, expert-parallel over 8 cores.
#
# v2 strategy (bf16 expert path, f-half slots):
#   - Router (Linear->ReLU->Linear, top-4 softmax) computed data-parallel in
#     fp32 on each core's 256-token shard; AllGather of the dense combine
#     weights. x^T for the router is pre-transposed on the host (xshT input).
#   - Expert work is decomposed into (expert, f-half, token-range) pieces:
#     each slot carries a 2048-wide f-slice of one expert (W1[:, fr], W2[fr, :])
#     so per-core weight traffic is NSLOT*2048 columns instead of NSLOT full
#     experts.  Partial products over f-halves add up in the output
#     accumulator, so halves combine for free.  Host packs pieces into a
#     uniform per-position capacity profile found by search (min total cap).
#   - All expert-path tensors are bf16 (weights, gathered x, h, y, output
#     accumulator, ReduceScatter); PSUM accumulation stays fp32.  Weights are
#     pre-arranged on the host for large contiguous DMAs.
#   - Dispatch per slot: expert mask -> exclusive cumsum (PE matmuls against
#     triangular constants) -> gate to [lo, lo+cap) -> ONE batched indirect-DMA
#     scatter of token ids -> batched gathers of token rows (bf16).
#   - mm1: h^T = gelu(W1e^T x^T + b1) per 512-col f-group; mm2 with W2 slice
#     SBUF-resident; y scaled by combine weight, scatter-added (CCE add, bf16)
#     into a dense [T,H] accumulator split in two column halves.
#   - ReduceScatter(add, bf16) per half over 8 cores, first half overlapped
#     with second-half compute; each core emits its 256-token fp32 shard.
import numpy as np

H = 1024
F = 4096
FH = 2048                  # f-slice width per slot (half of F)
E = 16
TOPK = 4
T = 2048
NCORES = 8
TSH = T // NCORES          # 256 router tokens per core
DUMP = T                   # dump token row index (row T of the [T+1] buffers)
NEG = -3.0e38
MARGIN = 16                # slack over host-computed counts (host/device drift)
HH = 512                   # output column half width

_CACHE = {}


# ---------------------------------------------------------------------------
# Host-side planning
# ---------------------------------------------------------------------------

def _host_counts(inputs):
    x = np.asarray(inputs["x"], np.float32).reshape(T, H)
    h = np.maximum(x @ np.asarray(inputs["Wr1"], np.float32)
                   + np.asarray(inputs["br1"], np.float32), 0.0)
    lg = h @ np.asarray(inputs["Wr2"], np.float32) + np.asarray(inputs["br2"], np.float32)
    order = np.argsort(-lg, axis=1, kind="stable")[:, :TOPK]
    counts = np.zeros(E, np.int64)
    for e in range(E):
        counts[e] = (order == e).sum()
    return counts


def _try_pack(piece_caps, profile):
    """piece_caps: list of ((e, half), cap) f-half pieces, splittable in token
    ranges.  profile: per-core position caps (each cap has NCORES positions).
    Returns asg[j][core] = ((e, half), lo) or None if infeasible."""
    avail = {}
    for c in profile:
        avail[c] = avail.get(c, 0) + NCORES
    sizes = sorted(set(profile), reverse=True)
    pieces = []
    for key, cap in sorted(piece_caps, key=lambda kc: -kc[1]):
        rem = cap
        lo = 0
        while rem > 0:
            pick = None
            for a in reversed(sizes):        # best fit
                if avail.get(a, 0) > 0 and a >= rem:
                    pick = a
                    break
            if pick is None:
                for a in sizes:              # largest available
                    if avail.get(a, 0) > 0:
                        pick = a
                        break
            if pick is None:
                return None
            avail[pick] -= 1
            pieces.append((pick, key, lo))
            lo += pick
            rem -= pick
    by_cap = {}
    for pc, key, lo in pieces:
        by_cap.setdefault(pc, []).append((key, lo))
    used = {c: 0 for c in set(profile)}
    out = []
    for c in profile:
        pos = []
        for r in range(NCORES):
            i = used[c]
            if i < len(by_cap.get(c, [])):
                pos.append(by_cap[c][i])
            else:
                pos.append(((0, 0), T + 4096))   # empty position
            used[c] += 1
        out.append(pos)
    return out


def _plan(inputs):
    from itertools import combinations_with_replacement

    counts = _host_counts(inputs)
    caps = [int(np.ceil((int(c) + MARGIN) / 128) * 128) for c in counts]
    piece_caps = []
    for e in range(E):
        for hf in range(2):
            piece_caps.append(((e, hf), caps[e]))
    menu = [1024, 896, 768, 640, 512, 384, 256, 128]
    best = None
    for ns in (4, 5, 6):
        for prof in combinations_with_replacement(menu, ns):
            s = sum(prof)
            if not (2 * sum(caps) // NCORES <= s <= 3072):
                continue
            asg = _try_pack(piece_caps, list(prof))
            if asg is not None:
                score = (s, ns)
                if best is None or score < best[0]:
                    best = (score, list(prof), asg)
    if best is None:
        raise RuntimeError(f"no feasible profile for counts {counts}")
    _, prof, asg = best
    return prof, asg


# ---------------------------------------------------------------------------
# Device program
# ---------------------------------------------------------------------------

def _build(profile):
    import concourse.bass as bass
    import concourse.mybir as mybir
    import concourse.tile as tile
    from concourse import bacc
    from concourse.masks import make_identity

    dt = mybir.dt
    f32 = dt.float32
    bf16 = dt.bfloat16
    i32 = dt.int32
    Alu = mybir.AluOpType
    Act = mybir.ActivationFunctionType
    NSLOT = len(profile)
    CMAX = max(profile)
    NFG = FH // 512            # 512-col f-groups per slot (4)
    NFC = FH // 128            # 128-row f-chunks per slot (16)

    nc = bacc.Bacc(None, target_bir_lowering=False, debug=False, num_devices=NCORES)

    # ---------------- I/O ----------------
    xfull16 = nc.dram_tensor("xfull16", [T, H], bf16, kind="ExternalInput")
    xshT = nc.dram_tensor("xshT", [128, H // 128, TSH], f32, kind="ExternalInput")
    Wr1 = nc.dram_tensor("Wr1", [H, H], f32, kind="ExternalInput")
    br1 = nc.dram_tensor("br1", [H], f32, kind="ExternalInput")
    Wr2 = nc.dram_tensor("Wr2", [H, E], f32, kind="ExternalInput")
    br2 = nc.dram_tensor("br2", [E], f32, kind="ExternalInput")
    # weights pre-arranged for big contiguous DMAs (see _in_maps)
    W1L = nc.dram_tensor("W1L", [NSLOT, NFG, 128, 8 * 512], bf16, kind="ExternalInput")
    W2L = nc.dram_tensor("W2L", [NSLOT, 128, NFC, H], bf16, kind="ExternalInput")
    b1L = nc.dram_tensor("b1L", [NSLOT, FH], f32, kind="ExternalInput")
    b2L = nc.dram_tensor("b2L", [NSLOT, H], bf16, kind="ExternalInput")
    ohL = nc.dram_tensor("ohL", [NSLOT, E], f32, kind="ExternalInput")
    slotlo = nc.dram_tensor("slotlo", [NSLOT], f32, kind="ExternalInput")
    out_sh = nc.dram_tensor("out_sh", [TSH, H], f32, kind="ExternalOutput")

    # ---------------- constants ----------------
    u128 = nc.inline_tensor(np.triu(np.ones((128, 128), np.float32), 1), "u128")
    u16 = nc.inline_tensor(np.triu(np.ones((16, 16), np.float32), 1), "u16")
    ones128 = nc.inline_tensor(np.ones((128, 1), np.float32), "ones128")
    tokid_np = (np.arange(16)[None, :] * 128 + np.arange(128)[:, None]).astype(np.int32)
    tokid = nc.inline_tensor(tokid_np, "tokid")
    idxinit = nc.inline_tensor(np.full((CMAX + 1, 1), DUMP, np.int32), "idxinit")

    # ---------------- internal DRAM ----------------
    c2aug = nc.dram_tensor("c2aug", [T + 1, NSLOT], f32)
    # 4 independent scatter targets per slot: splits the token-id scatter's
    # write-after-write chain into 4 parallel chains of 4 ops
    NGRP = 4
    idxbg = [[nc.dram_tensor(f"idxbg{k}_{g}", [profile[k] + 1, 1], i32)
              for g in range(NGRP)] for k in range(NSLOT)]
    outp2 = [nc.dram_tensor(f"outp{h}", [T + 1, HH], bf16) for h in range(2)]
    agin = nc.dram_tensor("agin", [TSH, E], f32)
    call = nc.dram_tensor("call", [T, E], f32, addr_space="Shared")
    rsout2 = [nc.dram_tensor(f"rsout{h}", [TSH, HH], bf16) for h in range(2)]

    RG = [list(range(NCORES))]

    with tile.TileContext(nc, pool_alloc_mode="queue") as tc:
        with (
            tc.tile_pool(name="const", bufs=1) as constp,
            tc.tile_pool(name="persist", bufs=1) as persist,
        ):
            ident16 = constp.tile([128, 128], bf16)
            make_identity(nc, ident16)
            u128_sb = constp.tile_from(u128.ap())
            u16_sb = constp.tile_from(u16.ap())
            ones128_sb = constp.tile_from(ones128.ap())
            tokid_sb = constp.tile_from(tokid.ap())
            onesmm_f32 = constp.tile([1, 128], f32)
            nc.vector.memset(onesmm_f32[:], 1.0)
            onesmm_sb = constp.tile([1, 128], bf16)
            nc.vector.tensor_copy(onesmm_sb[:], onesmm_f32[:])
            zero_sb = constp.tile([128, HH], bf16)
            nc.vector.memset(zero_sb[:], 0.0)
            zero_f32 = constp.tile([1, NSLOT], f32)
            nc.vector.memset(zero_f32[:], 0.0)

            # ====== phase 1: router on this core's 256-token shard (fp32) ======
            with (
                tc.tile_pool(name="rweights", bufs=1) as rw,
                tc.tile_pool(name="rtmp", bufs=3) as rt,
                tc.tile_pool(name="rpsum", bufs=2, space="PSUM") as rp,
            ):
                xt_sh = rw.tile([128, H // 128, TSH], f32)
                nc.sync.dma_start(xt_sh[:], xshT.ap())
                # stream Wr1 per 128-col output block so mm1 starts early
                # (small per-ho tiles keep the router pool footprint low)
                wr1_t = []
                for ho in range(H // 128):
                    t = rw.tile([128, H // 128, 128], f32, tag=f"wr1_{ho}",
                                name=f"wr1_{ho}")
                    nc.sync.dma_start(
                        t[:],
                        Wr1.ap()[:, ho * 128:(ho + 1) * 128]
                        .rearrange("(c p) o -> p c o", p=128))
                    wr1_t.append(t)
                wr2_sb = rw.tile([128, H // 128, E], f32)
                nc.sync.dma_start(wr2_sb[:], Wr2.ap().rearrange("(c p) e -> p c e", p=128))
                br1_sb = rw.tile([128, H // 128], f32)
                nc.sync.dma_start(br1_sb[:], br1.ap().rearrange("(c p) -> p c", p=128))
                br2_rep = rw.tile([128, E], f32)
                nc.sync.dma_start(
                    br2_rep[:],
                    br2.ap().rearrange("(o e) -> o e", o=1).to_broadcast([128, E]))

                r1t = rw.tile([128, H // 128, TSH], f32)
                for ho in range(H // 128):
                    p1 = rp.tile([128, TSH], f32, tag="p1")
                    for hc in range(H // 128):
                        nc.tensor.matmul(
                            p1[:], wr1_t[ho][:, hc, :], xt_sh[:, hc, :],
                            start=(hc == 0), stop=(hc == H // 128 - 1))
                    nc.scalar.activation(r1t[:, ho, :], p1[:], Act.Relu,
                                         bias=br1_sb[:, ho:ho + 1])

                for t2 in range(TSH // 128):
                    p2 = rp.tile([128, E], f32, tag="p2")
                    for hc in range(H // 128):
                        nc.tensor.matmul(
                            p2[:], r1t[:, hc, t2 * 128:(t2 + 1) * 128], wr2_sb[:, hc, :],
                            start=(hc == 0), stop=(hc == H // 128 - 1))
                    lg = rt.tile([128, E], f32, tag="lg")
                    nc.vector.tensor_tensor(lg[:], p2[:], br2_rep[:], op=Alu.add)
                    mx8 = rt.tile([128, 8], f32, tag="mx8")
                    nc.vector.max(mx8[:], lg[:])
                    mx4 = rt.tile([128, 8], f32, tag="mx4")
                    nc.vector.memset(mx4[:], NEG)
                    nc.vector.tensor_copy(mx4[:, 0:TOPK], mx8[:, 0:TOPK])
                    zap = rt.tile([128, E], f32, tag="zap")
                    nc.vector.match_replace(zap[:], in_to_replace=mx4[:], in_values=lg[:],
                                            imm_value=NEG)
                    mask = rt.tile([128, E], f32, tag="mask")
                    nc.vector.tensor_tensor(mask[:], lg[:], zap[:], op=Alu.not_equal)
                    negmax = rt.tile([128, 1], f32, tag="negmax")
                    nc.vector.tensor_scalar_mul(negmax[:], mx8[:, 0:1], -1.0)
                    ex = rt.tile([128, E], f32, tag="ex")
                    nc.scalar.activation(ex[:], lg[:], Act.Exp, bias=negmax[:])
                    nc.vector.tensor_tensor(ex[:], ex[:], mask[:], op=Alu.mult)
                    den = rt.tile([128, 1], f32, tag="den")
                    nc.vector.reduce_sum(den[:], ex[:], axis=mybir.AxisListType.X)
                    rcp = rt.tile([128, 1], f32, tag="rcp")
                    nc.vector.reciprocal(rcp[:], den[:])
                    csh = rt.tile([128, E], f32, tag="csh")
                    nc.vector.tensor_scalar(csh[:], ex[:], rcp[:], None, op0=Alu.mult)
                    nc.sync.dma_start(agin[t2 * 128:(t2 + 1) * 128, :], csh[:])

            ohrep = persist.tile([128, NSLOT, E], f32, tag="ohrep")
            nc.sync.dma_start(
                ohrep[:],
                ohL.ap().rearrange("(o l) e -> o l e", o=1).to_broadcast([128, NSLOT, E]))
            lo_rep = persist.tile([128, NSLOT], f32, tag="lo_rep")
            nc.sync.dma_start(
                lo_rep[:],
                slotlo.ap().rearrange("(o l) -> o l", o=1).to_broadcast([128, NSLOT]))

            nc.gpsimd.collective_compute(
                "AllGather", Alu.bypass, replica_groups=RG,
                ins=[agin.ap().opt()], outs=[call.ap().opt()])

            # deferred init (Activation queue is idle while the router computes)
            for k in range(NSLOT):
                for g in range(NGRP):
                    nc.scalar.dma_start(idxbg[k][g][:],
                                        idxinit.ap()[0:profile[k] + 1, :])
            nc.sync.dma_start(c2aug[T:T + 1, :], zero_f32[:])
            for h in range(2):
                for k in range(T // 128):
                    nc.sync.dma_start(outp2[h][k * 128:(k + 1) * 128, :], zero_sb[:])
                nc.sync.dma_start(outp2[h][T:T + 1, :], zero_sb[0:1, :])

            # ====== phase 2: dispatch for the NSLOT local slots ======
            idx_sb = []
            s_col = []
            with (
                tc.tile_pool(name="dsb", bufs=3) as dsb,
                tc.tile_pool(name="dps", bufs=2, space="PSUM") as dps,
            ):
                cf = persist.tile([128, T // 128, E], f32, tag="cfall")
                nc.sync.dma_start(cf[:], call.ap().rearrange("(c p) e -> p c e", p=128))
                c2sb = persist.tile([128, T // 128, NSLOT], f32)
                xg_hold = [persist.tile([128, profile[k] // 128, H], bf16,
                                        tag=f"xgh{k}", name=f"xgh{k}")
                           for k in range(NSLOT)]
                for k in range(NSLOT):
                    idx_sb.append(persist.tile([128, profile[k] // 128], i32,
                                               tag=f"idx{k}", name=f"idx{k}"))
                    s_col.append(persist.tile([128, profile[k] // 128], f32,
                                              tag=f"scol{k}", name=f"scol{k}"))

                # pre-pass: per-slot combine weights ce -> c2sb, then one
                # c2aug store (needed by all scol gathers)
                for k in range(NSLOT):
                    msk = dsb.tile([128, T // 128, E], f32, tag="msk")
                    nc.vector.tensor_tensor(
                        msk[:], cf[:],
                        ohrep[:, k:k + 1, :].to_broadcast([128, T // 128, E]),
                        op=Alu.mult)
                    ce = dsb.tile([128, T // 128], f32, tag="ce")
                    nc.vector.reduce_sum(ce[:], msk[:], axis=mybir.AxisListType.X)
                    nc.vector.tensor_copy(c2sb[:, :, k], ce[:])
                nc.sync.dma_start(
                    c2aug.ap()[0:T, :].rearrange("(c p) l -> p c l", p=128),
                    c2sb[:])

                for k in range(NSLOT):
                    A = profile[k]
                    m = dsb.tile([128, T // 128], f32, tag="m")
                    nc.vector.tensor_scalar(m[:], c2sb[:, :, k], 0.0, None,
                                            op0=Alu.not_equal)

                    # exclusive cumsum over global token order
                    csp = dps.tile([16, 1], f32, tag="csp")
                    nc.tensor.matmul(csp[:], m[:], ones128_sb[:], start=True, stop=True)
                    cs_sb = dsb.tile([16, 1], f32, tag="cs_sb")
                    nc.any.tensor_copy(cs_sb[:], csp[:])
                    csrep = dsb.tile([16, 128], f32, tag="csrep")
                    nc.vector.tensor_copy(csrep[:], cs_sb[:].to_broadcast([16, 128]))
                    posp = dps.tile([128, T // 128], f32, tag="posp")
                    nc.tensor.matmul(posp[:], u128_sb[:], m[:], start=True, stop=False)
                    nc.tensor.matmul(posp[:], csrep[:], u16_sb[:], start=False, stop=True)

                    # gate to [lo, lo+A)
                    tpos = dsb.tile([128, T // 128], f32, tag="tpos")
                    nc.vector.tensor_scalar(tpos[:], posp[:], lo_rep[:, k:k + 1], None,
                                            op0=Alu.subtract)
                    g1 = dsb.tile([128, T // 128], f32, tag="g1")
                    nc.vector.tensor_scalar(g1[:], tpos[:], 0.0, None, op0=Alu.is_ge)
                    g2 = dsb.tile([128, T // 128], f32, tag="g2")
                    nc.vector.tensor_scalar(g2[:], tpos[:], float(A), None, op0=Alu.is_lt)
                    nc.vector.tensor_tensor(m[:], m[:], g1[:], op=Alu.mult)
                    nc.vector.tensor_tensor(m[:], m[:], g2[:], op=Alu.mult)

                    # offsets: O = A + m*(tpos - A)   (unselected -> dump slot A)
                    of = dsb.tile([128, T // 128], f32, tag="of")
                    nc.vector.tensor_scalar(of[:], tpos[:], float(A), None, op0=Alu.subtract)
                    nc.vector.tensor_tensor(of[:], of[:], m[:], op=Alu.mult)
                    nc.vector.tensor_scalar(of[:], of[:], float(A), None, op0=Alu.add)
                    oi = dsb.tile([128, T // 128], i32, tag="oi")
                    nc.vector.tensor_copy(oi[:], of[:])

                    # scatter token ids (single-column [128,1] offset APs; 4
                    # independent target tensors so the WAW chains run in
                    # parallel)
                    for g in range(NGRP):
                        for c in range(T // 128 // NGRP):
                            cc = g * (T // 128 // NGRP) + c
                            nc.gpsimd.indirect_dma_start(
                                out=idxbg[k][g].ap(),
                                out_offset=bass.IndirectOffsetOnAxis(
                                    ap=oi[:, cc:cc + 1], axis=0),
                                in_=tokid_sb[:, cc:cc + 1], in_offset=None,
                                bounds_check=A, oob_is_err=False)

                    # merge the 4 scatter groups (min; DUMP is the identity)
                    # and this slot's gathers, all slot-local and all on the
                    # Pool/DVE queues so later slots never block earlier ones
                    NCK = A // 128
                    tg = []
                    for g in range(NGRP):
                        t = dsb.tile([128, CMAX // 128], i32, tag=f"tg{g}")
                        nc.gpsimd.dma_start(
                            t[:, 0:NCK],
                            idxbg[k][g].ap()[0:A, :].rearrange("(c p) o -> p (c o)", p=128))
                        tg.append(t)
                    # tree min-reduce of the NGRP scatter groups
                    stride = 1
                    while stride < NGRP:
                        for g in range(0, NGRP, 2 * stride):
                            nc.vector.tensor_tensor(
                                tg[g][:, 0:NCK], tg[g][:, 0:NCK],
                                tg[g + stride][:, 0:NCK], op=Alu.min)
                        stride *= 2
                    nc.vector.tensor_copy(idx_sb[k][:], tg[0][:, 0:NCK])
                    for ck in range(NCK):
                        nc.gpsimd.indirect_dma_start(
                            out=xg_hold[k][:, ck, :], out_offset=None,
                            in_=xfull16.ap(),
                            in_offset=bass.IndirectOffsetOnAxis(
                                ap=idx_sb[k][:, ck:ck + 1], axis=0),
                            bounds_check=T - 1, oob_is_err=False)
                    for ck in range(NCK):
                        nc.gpsimd.indirect_dma_start(
                            out=s_col[k][:, ck:ck + 1], out_offset=None,
                            in_=c2aug.ap(),
                            in_offset=bass.IndirectOffsetOnAxis(
                                ap=idx_sb[k][:, ck:ck + 1], axis=0),
                            element_offset=k,
                            bounds_check=T, oob_is_err=True)

            # ====== phase 3: expert MLP per slot ======
            with (
                tc.tile_pool(name="w1", bufs=3) as w1p,
                tc.tile_pool(name="w2", bufs=1) as w2p,
                tc.tile_pool(name="hbuf", bufs=1) as hbp,
                tc.tile_pool(name="xt", bufs=2) as xtp,
                tc.tile_pool(name="ysb", bufs=2) as ysp,
                tc.tile_pool(name="bias", bufs=1) as biasp,
                tc.tile_pool(name="psh", bufs=3, space="PSUM") as psh,
                tc.tile_pool(name="psy", bufs=4, space="PSUM") as psy,
                tc.tile_pool(name="pst", bufs=1, space="PSUM") as pst,
            ):
                b1_sb = biasp.tile([128, NSLOT, NFC], f32)
                nc.sync.dma_start(b1_sb[:], b1L.ap().rearrange("l (c p) -> p l c", p=128))

                for k in range(NSLOT):
                    A = profile[k]
                    NCK = A // 128
                    # transpose gathered x rows on the PE (the HW serializes
                    # xbar DMA-transposes against other SBUF DMAs, which
                    # starves the weight stream — measured slower than PE)
                    xt = xtp.tile([128, H // 128, CMAX], bf16, tag="xt")
                    for ck in range(NCK):
                        for hc in range(H // 128):
                            tp = pst.tile([128, 128], bf16, tag="tp3")
                            nc.tensor.transpose(
                                tp[:], xg_hold[k][:, ck, hc * 128:(hc + 1) * 128],
                                ident16[:])
                            nc.any.tensor_copy(
                                xt[:, hc, ck * 128:(ck + 1) * 128], tp[:])

                    b2_sb = biasp.tile([1, H], bf16, tag="b2_sb")
                    nc.sync.dma_start(b2_sb[:], b2L.ap()[k:k + 1, :])

                    # mm1: h^T[f, c] = gelu(sum_h W1[h,f]^T x^T[h,c] + b1[f])
                    chs = [A] if A <= 512 else ([A // 2, A // 2] if A <= 1024
                                                else [512, 512, A - 1024])
                    hbuf = hbp.tile([128, NFC, CMAX], bf16, tag="hbuf")
                    for fo in range(NFG):
                        w1f = w1p.tile([128, 8, 512], bf16, tag="w1f")
                        nc.sync.dma_start(
                            w1f[:], W1L[k, fo].rearrange("p (c f) -> p c f", c=8))
                        for fi in range(4):
                            fg = fo * 4 + fi
                            cc0 = 0
                            for ch in chs:
                                ph = psh.tile([128, 512], f32, tag="ph")
                                for hc in range(H // 128):
                                    nc.tensor.matmul(
                                        ph[:, 0:ch],
                                        w1f[:, hc, fi * 128:(fi + 1) * 128],
                                        xt[:, hc, cc0:cc0 + ch],
                                        start=(hc == 0), stop=(hc == H // 128 - 1))
                                nc.scalar.activation(
                                    hbuf[:, fg, cc0:cc0 + ch], ph[:, 0:ch],
                                    Act.Gelu, bias=b1_sb[:, k, fg:fg + 1])
                                cc0 += ch

                    # W2 slice resident for the slot; the DMA overlaps the
                    # mm1 tail (emitted after mm1's w1f loads on the SP queue)
                    w2r = w2p.tile([128, NFC, H], bf16, tag="w2r")
                    nc.sync.dma_start(w2r[:], W2L[k])

                    # mm2: y[c, h] = (sum_f h^T[f,c]^T W2[f,h] + b2[h]) * s[c]
                    for hh in range(2):
                        for tb0 in range(0, NCK, 4):
                            tbn = min(4, NCK - tb0)
                            pys = [psy.tile([128, HH], f32, tag="py", name=f"py{_i}")
                                   for _i in range(tbn)]
                            for fc in range(NFC):
                                for i in range(tbn):
                                    ck = tb0 + i
                                    nc.tensor.matmul(
                                        pys[i][:],
                                        hbuf[:, fc, ck * 128:(ck + 1) * 128],
                                        w2r[:, fc, hh * HH:(hh + 1) * HH],
                                        start=(fc == 0), stop=False)
                            ysb = ysp.tile([128, 4, HH], bf16, tag="ysb")
                            for i in range(tbn):
                                ck = tb0 + i
                                nc.tensor.matmul(
                                    pys[i][:], onesmm_sb[0:1, :],
                                    b2_sb[0:1, hh * HH:(hh + 1) * HH],
                                    start=False, stop=True)
                                nc.scalar.activation(
                                    ysb[:, i, :], pys[i][:], Act.Copy,
                                    scale=s_col[k][:, ck:ck + 1])
                                nc.gpsimd.indirect_dma_start(
                                    out=outp2[hh].ap(),
                                    out_offset=bass.IndirectOffsetOnAxis(
                                        ap=idx_sb[k][:, ck:ck + 1], axis=0),
                                    in_=ysb[:, i, :], in_offset=None,
                                    compute_op=Alu.add,
                                    bounds_check=T, oob_is_err=True)
                        if k == NSLOT - 1 and hh == 0:
                            # all h-half-0 contributions are in: overlap its
                            # ReduceScatter with h-half-1 compute
                            nc.gpsimd.collective_compute(
                                "ReduceScatter", Alu.add, replica_groups=RG,
                                ins=[outp2[0].ap()[0:T, :].opt()],
                                outs=[rsout2[0].ap().opt()])

            # ====== phase 4: remaining reduce + output shard ======
            with tc.tile_pool(name="outc", bufs=2) as outc:
                for k in range(TSH // 128):
                    ot = outc.tile([128, HH], bf16, tag="ot")
                    nc.sync.dma_start(ot[:], rsout2[0][k * 128:(k + 1) * 128, :])
                    otf = outc.tile([128, HH], f32, tag="otf")
                    nc.vector.tensor_copy(otf[:], ot[:])
                    nc.sync.dma_start(out_sh[k * 128:(k + 1) * 128, 0:HH], otf[:])
                nc.gpsimd.collective_compute(
                    "ReduceScatter", Alu.add, replica_groups=RG,
                    ins=[outp2[1].ap()[0:T, :].opt()], outs=[rsout2[1].ap().opt()])
                for k in range(TSH // 128):
                    ot = outc.tile([128, HH], bf16, tag="ot")
                    nc.sync.dma_start(ot[:], rsout2[1][k * 128:(k + 1) * 128, :])
                    otf = outc.tile([128, HH], f32, tag="otf")
                    nc.vector.tensor_copy(otf[:], ot[:])
                    nc.sync.dma_start(out_sh[k * 128:(k + 1) * 128, HH:H], otf[:])

    nc.compile()
    if not nc.is_finalized():
        nc.finalize()
    return nc


# ---------------------------------------------------------------------------
# Host-side input preparation
# ---------------------------------------------------------------------------

def _in_maps(inputs, profile, asg):
    import ml_dtypes
    bf16 = ml_dtypes.bfloat16

    NSLOT = len(profile)
    NFG = FH // 512
    NFC = FH // 128
    x = np.ascontiguousarray(np.asarray(inputs["x"], np.float32).reshape(T, H))
    W1 = np.asarray(inputs["W1"], np.float32)
    b1 = np.asarray(inputs["b1"], np.float32)
    W2 = np.asarray(inputs["W2"], np.float32)
    b2 = np.asarray(inputs["b2"], np.float32)
    common = {
        "xfull16": np.ascontiguousarray(x.astype(bf16)),
        "Wr1": np.ascontiguousarray(np.asarray(inputs["Wr1"], np.float32)),
        "br1": np.ascontiguousarray(np.asarray(inputs["br1"], np.float32)),
        "Wr2": np.ascontiguousarray(np.asarray(inputs["Wr2"], np.float32)),
        "br2": np.ascontiguousarray(np.asarray(inputs["br2"], np.float32)),
    }
    maps = []
    for r in range(NCORES):
        w1l = np.empty((NSLOT, NFG, 128, 8 * 512), bf16)
        w2l = np.empty((NSLOT, 128, NFC, H), bf16)
        b1l = np.zeros((NSLOT, FH), np.float32)
        b2l = np.zeros((NSLOT, H), bf16)
        oh = np.zeros((NSLOT, E), np.float32)
        lo = np.zeros((NSLOT,), np.float32)
        for k in range(NSLOT):
            (e, hf), l0 = asg[k][r]
            f0 = hf * FH
            w1h = W1[e][:, f0:f0 + FH]                    # [H, FH]
            # W1L[k, fo, p, hc*512+fc] = w1h[hc*128+p, fo*512+fc]
            w1l[k] = (w1h.reshape(8, 128, NFG, 512)       # hc, p, fo, fc
                      .transpose(2, 1, 0, 3)              # fo, p, hc, fc
                      .reshape(NFG, 128, 8 * 512).astype(bf16))
            w2h = W2[e][f0:f0 + FH, :]                    # [FH, H]
            # W2L[k, p, fc, ho] = w2h[fc*128+p, ho]
            w2l[k] = (w2h.reshape(NFC, 128, H)
                      .transpose(1, 0, 2).astype(bf16))
            b1l[k] = b1[e][f0:f0 + FH]
            if hf == 0:
                b2l[k] = b2[e].astype(bf16)
            if l0 <= T:
                oh[k, e] = 1.0       # empty slots keep an all-zero one-hot
            lo[k] = float(l0)
        # shard x^T for the router: xshT[p, hc, t] = x[r*TSH+t, hc*128+p]
        xs = x[r * TSH:(r + 1) * TSH]                     # [TSH, H]
        xshT = np.ascontiguousarray(
            xs.T.reshape(8, 128, TSH).transpose(1, 0, 2))
        maps.append({
            **common,
            "xshT": xshT,
            "W1L": w1l, "W2L": w2l, "b1L": b1l, "b2L": b2l,
            "ohL": oh, "slotlo": lo,
        })
    return maps


def _get_nc(profile):
    key = tuple(profile)
    if key not in _CACHE:
        _CACHE[key] = _build(list(key))
    return _CACHE[key]


def kernel(**inputs) -> np.ndarray:
    from concourse.bass_utils import run_bass_kernel_spmd

    profile, asg = _plan(inputs)
    nc = _get_nc(profile)
    maps = _in_maps(inputs, profile, asg)
    res = run_bass_kernel_spmd(nc, maps, core_ids=list(range(NCORES)))
    shards = [res.results[r]["out_sh"] for r in range(NCORES)]
    out = np.concatenate(shards, axis=0).reshape(np.asarray(inputs["x"]).shape)
    return out.astype(np.float32)


# revision 30
# speedup vs baseline: 1.9597x; 1.2655x over previous
# kernel.py — MoE (E=16, top-4) Trainium2 Bass kernel, expert-parallel over 8 cores.
#
# v2 strategy (bf16 expert path, f-half slots):
#   - Router (Linear->ReLU->Linear, top-4 softmax) computed data-parallel in
#     fp32 on each core's 256-token shard; AllGather of the dense combine
#     weights. x^T for the router is pre-transposed on the host (xshT input).
#   - Expert work is decomposed into (expert, f-half, token-range) pieces:
#     each slot carries a 2048-wide f-slice of one expert (W1[:, fr], W2[fr, :])
#     so per-core weight traffic is NSLOT*2048 columns instead of NSLOT full
#     experts.  Partial products over f-halves add up in the output
#     accumulator, so halves combine for free.  Host packs pieces into a
#     uniform per-position capacity profile found by search (min total cap).
#   - All expert-path tensors are bf16 (weights, gathered x, h, y, output
#     accumulator, ReduceScatter); PSUM accumulation stays fp32.  Weights are
#     pre-arranged on the host for large contiguous DMAs.
#   - Dispatch per slot: expert mask -> exclusive cumsum (PE matmuls against
#     triangular constants) -> gate to [lo, lo+cap) -> ONE batched indirect-DMA
#     scatter of token ids -> batched gathers of token rows (bf16).
#   - mm1: h^T = gelu(W1e^T x^T + b1) per 512-col f-group; mm2 with W2 slice
#     SBUF-resident; y scaled by combine weight, scatter-added (CCE add, bf16)
#     into a dense [T,H] accumulator split in two column halves.
#   - ReduceScatter(add, bf16) per half over 8 cores, first half overlapped
#     with second-half compute; each core emits its 256-token fp32 shard.
import numpy as np

H = 1024
F = 4096
FH = 2048                  # f-slice width per slot (half of F)
E = 16
TOPK = 4
T = 2048
NCORES = 8
TSH = T // NCORES          # 256 router tokens per core
DUMP = T                   # dump token row index (row T of the [T+1] buffers)
NEG = -3.0e38
MARGIN = 16                # slack over host-computed counts (host/device drift)
HH = 512                   # output column half width

_CACHE = {}


# ---------------------------------------------------------------------------
# Host-side planning
# ---------------------------------------------------------------------------

def _host_counts(inputs):
    x = np.asarray(inputs["x"], np.float32).reshape(T, H)
    h = np.maximum(x @ np.asarray(inputs["Wr1"], np.float32)
                   + np.asarray(inputs["br1"], np.float32), 0.0)
    lg = h @ np.asarray(inputs["Wr2"], np.float32) + np.asarray(inputs["br2"], np.float32)
    order = np.argsort(-lg, axis=1, kind="stable")[:, :TOPK]
    counts = np.zeros(E, np.int64)
    for e in range(E):
        counts[e] = (order == e).sum()
    return counts


def _try_pack(piece_caps, profile):
    """piece_caps: list of ((e, half), cap) f-half pieces, splittable in token
    ranges.  profile: per-core position caps (each cap has NCORES positions).
    Returns asg[j][core] = ((e, half), lo) or None if infeasible."""
    avail = {}
    for c in profile:
        avail[c] = avail.get(c, 0) + NCORES
    sizes = sorted(set(profile), reverse=True)
    pieces = []
    for key, cap in sorted(piece_caps, key=lambda kc: -kc[1]):
        rem = cap
        lo = 0
        while rem > 0:
            pick = None
            for a in reversed(sizes):        # best fit
                if avail.get(a, 0) > 0 and a >= rem:
                    pick = a
                    break
            if pick is None:
                for a in sizes:              # largest available
                    if avail.get(a, 0) > 0:
                        pick = a
                        break
            if pick is None:
                return None
            avail[pick] -= 1
            pieces.append((pick, key, lo))
            lo += pick
            rem -= pick
    by_cap = {}
    for pc, key, lo in pieces:
        by_cap.setdefault(pc, []).append((key, lo))
    used = {c: 0 for c in set(profile)}
    out = []
    for c in profile:
        pos = []
        for r in range(NCORES):
            i = used[c]
            if i < len(by_cap.get(c, [])):
                pos.append(by_cap[c][i])
            else:
                pos.append(((0, 0), T + 4096))   # empty position
            used[c] += 1
        out.append(pos)
    return out


def _plan(inputs):
    from itertools import combinations_with_replacement

    counts = _host_counts(inputs)
    caps = [int(np.ceil((int(c) + MARGIN) / 128) * 128) for c in counts]
    piece_caps = []
    for e in range(E):
        for hf in range(2):
            piece_caps.append(((e, hf), caps[e]))
    menu = [1024, 896, 768, 640, 512, 384, 256, 128]
    best = None
    for ns in (5,):
        for prof in combinations_with_replacement(menu, ns):
            s = sum(prof)
            if not (2 * sum(caps) // NCORES <= s <= 3072):
                continue
            asg = _try_pack(piece_caps, list(prof))
            if asg is not None:
                score = (s, ns)
                if best is None or score < best[0]:
                    best = (score, list(prof), asg)
    if best is None:
        raise RuntimeError(f"no feasible profile for counts {counts}")
    _, prof, asg = best
    return prof, asg


# ---------------------------------------------------------------------------
# Device program
# ---------------------------------------------------------------------------

def _build(profile):
    import concourse.bass as bass
    import concourse.mybir as mybir
    import concourse.tile as tile
    from concourse import bacc
    from concourse.masks import make_identity

    dt = mybir.dt
    f32 = dt.float32
    bf16 = dt.bfloat16
    i32 = dt.int32
    Alu = mybir.AluOpType
    Act = mybir.ActivationFunctionType
    NSLOT = len(profile)
    CMAX = max(profile)
    NFG = FH // 512            # 512-col f-groups per slot (4)
    NFC = FH // 128            # 128-row f-chunks per slot (16)

    nc = bacc.Bacc(None, target_bir_lowering=False, debug=False, num_devices=NCORES)

    # ---------------- I/O ----------------
    xfull16 = nc.dram_tensor("xfull16", [T, H], bf16, kind="ExternalInput")
    xshT = nc.dram_tensor("xshT", [128, H // 128, TSH], f32, kind="ExternalInput")
    Wr1 = nc.dram_tensor("Wr1", [H, H], f32, kind="ExternalInput")
    br1 = nc.dram_tensor("br1", [H], f32, kind="ExternalInput")
    Wr2 = nc.dram_tensor("Wr2", [H, E], f32, kind="ExternalInput")
    br2 = nc.dram_tensor("br2", [E], f32, kind="ExternalInput")
    # weights pre-arranged for big contiguous DMAs (see _in_maps)
    W1L = nc.dram_tensor("W1L", [NSLOT, NFG, 128, 8 * 512], bf16, kind="ExternalInput")
    W2L = nc.dram_tensor("W2L", [NSLOT, 128, NFC, H], bf16, kind="ExternalInput")
    b1L = nc.dram_tensor("b1L", [NSLOT, FH], f32, kind="ExternalInput")
    b2L = nc.dram_tensor("b2L", [NSLOT, H], bf16, kind="ExternalInput")
    ohL = nc.dram_tensor("ohL", [NSLOT, E], f32, kind="ExternalInput")
    slotlo = nc.dram_tensor("slotlo", [NSLOT], f32, kind="ExternalInput")
    out_sh = nc.dram_tensor("out_sh", [TSH, H], f32, kind="ExternalOutput")

    # ---------------- constants ----------------
    u128 = nc.inline_tensor(np.triu(np.ones((128, 128), np.float32), 1), "u128")
    u16 = nc.inline_tensor(np.triu(np.ones((16, 16), np.float32), 1), "u16")
    ones128 = nc.inline_tensor(np.ones((128, 1), np.float32), "ones128")
    tokid_np = (np.arange(16)[None, :] * 128 + np.arange(128)[:, None]).astype(np.int32)
    tokid = nc.inline_tensor(tokid_np, "tokid")
    idxinit = nc.inline_tensor(np.full((CMAX + 1, 1), DUMP, np.int32), "idxinit")

    # ---------------- internal DRAM ----------------
    c2aug = nc.dram_tensor("c2aug", [T + 1, NSLOT], f32)
    # 4 independent scatter targets per slot: splits the token-id scatter's
    # write-after-write chain into 4 parallel chains of 4 ops
    NGRP = 4
    idxbg = [[nc.dram_tensor(f"idxbg{k}_{g}", [profile[k] + 1, 1], i32)
              for g in range(NGRP)] for k in range(NSLOT)]
    outp2 = [nc.dram_tensor(f"outp{h}", [T + 1, HH], bf16) for h in range(2)]
    agin = nc.dram_tensor("agin", [TSH, E], f32)
    call = nc.dram_tensor("call", [T, E], f32, addr_space="Shared")
    rsout2 = [nc.dram_tensor(f"rsout{h}", [TSH, HH], bf16) for h in range(2)]

    RG = [list(range(NCORES))]

    with tile.TileContext(nc, pool_alloc_mode="queue") as tc:
        with (
            tc.tile_pool(name="const", bufs=1) as constp,
            tc.tile_pool(name="persist", bufs=1) as persist,
        ):
            ident16 = constp.tile([128, 128], bf16)
            make_identity(nc, ident16)
            u128_sb = constp.tile_from(u128.ap())
            u16_sb = constp.tile_from(u16.ap())
            ones128_sb = constp.tile_from(ones128.ap())
            tokid_sb = constp.tile_from(tokid.ap())
            onesmm_f32 = constp.tile([1, 128], f32)
            nc.vector.memset(onesmm_f32[:], 1.0)
            onesmm_sb = constp.tile([1, 128], bf16)
            nc.vector.tensor_copy(onesmm_sb[:], onesmm_f32[:])
            zero_sb = constp.tile([128, HH], bf16)
            nc.vector.memset(zero_sb[:], 0.0)
            zero_f32 = constp.tile([1, NSLOT], f32)
            nc.vector.memset(zero_f32[:], 0.0)

            # ====== phase 1: router on this core's 256-token shard (fp32) ======
            with (
                tc.tile_pool(name="rweights", bufs=1) as rw,
                tc.tile_pool(name="rtmp", bufs=3) as rt,
                tc.tile_pool(name="rpsum", bufs=2, space="PSUM") as rp,
            ):
                xt_sh = rw.tile([128, H // 128, TSH], f32)
                nc.sync.dma_start(xt_sh[:], xshT.ap())
                # stream Wr1 per 128-col output block so mm1 starts early
                # (small per-ho tiles keep the router pool footprint low)
                wr1_t = []
                for ho in range(H // 128):
                    t = rw.tile([128, H // 128, 128], f32, tag=f"wr1_{ho}",
                                name=f"wr1_{ho}")
                    nc.sync.dma_start(
                        t[:],
                        Wr1.ap()[:, ho * 128:(ho + 1) * 128]
                        .rearrange("(c p) o -> p c o", p=128))
                    wr1_t.append(t)
                wr2_sb = rw.tile([128, H // 128, E], f32)
                nc.sync.dma_start(wr2_sb[:], Wr2.ap().rearrange("(c p) e -> p c e", p=128))
                br1_sb = rw.tile([128, H // 128], f32)
                nc.sync.dma_start(br1_sb[:], br1.ap().rearrange("(c p) -> p c", p=128))
                br2_rep = rw.tile([128, E], f32)
                nc.sync.dma_start(
                    br2_rep[:],
                    br2.ap().rearrange("(o e) -> o e", o=1).to_broadcast([128, E]))

                r1t = rw.tile([128, H // 128, TSH], f32)
                for ho in range(H // 128):
                    p1 = rp.tile([128, TSH], f32, tag="p1")
                    for hc in range(H // 128):
                        nc.tensor.matmul(
                            p1[:], wr1_t[ho][:, hc, :], xt_sh[:, hc, :],
                            start=(hc == 0), stop=(hc == H // 128 - 1))
                    nc.scalar.activation(r1t[:, ho, :], p1[:], Act.Relu,
                                         bias=br1_sb[:, ho:ho + 1])

                for t2 in range(TSH // 128):
                    p2 = rp.tile([128, E], f32, tag="p2")
                    for hc in range(H // 128):
                        nc.tensor.matmul(
                            p2[:], r1t[:, hc, t2 * 128:(t2 + 1) * 128], wr2_sb[:, hc, :],
                            start=(hc == 0), stop=(hc == H // 128 - 1))
                    lg = rt.tile([128, E], f32, tag="lg")
                    nc.vector.tensor_tensor(lg[:], p2[:], br2_rep[:], op=Alu.add)
                    mx8 = rt.tile([128, 8], f32, tag="mx8")
                    nc.vector.max(mx8[:], lg[:])
                    mx4 = rt.tile([128, 8], f32, tag="mx4")
                    nc.vector.memset(mx4[:], NEG)
                    nc.vector.tensor_copy(mx4[:, 0:TOPK], mx8[:, 0:TOPK])
                    zap = rt.tile([128, E], f32, tag="zap")
                    nc.vector.match_replace(zap[:], in_to_replace=mx4[:], in_values=lg[:],
                                            imm_value=NEG)
                    mask = rt.tile([128, E], f32, tag="mask")
                    nc.vector.tensor_tensor(mask[:], lg[:], zap[:], op=Alu.not_equal)
                    negmax = rt.tile([128, 1], f32, tag="negmax")
                    nc.vector.tensor_scalar_mul(negmax[:], mx8[:, 0:1], -1.0)
                    ex = rt.tile([128, E], f32, tag="ex")
                    nc.scalar.activation(ex[:], lg[:], Act.Exp, bias=negmax[:])
                    nc.vector.tensor_tensor(ex[:], ex[:], mask[:], op=Alu.mult)
                    den = rt.tile([128, 1], f32, tag="den")
                    nc.vector.reduce_sum(den[:], ex[:], axis=mybir.AxisListType.X)
                    rcp = rt.tile([128, 1], f32, tag="rcp")
                    nc.vector.reciprocal(rcp[:], den[:])
                    csh = rt.tile([128, E], f32, tag="csh")
                    nc.vector.tensor_scalar(csh[:], ex[:], rcp[:], None, op0=Alu.mult)
                    nc.sync.dma_start(agin[t2 * 128:(t2 + 1) * 128, :], csh[:])

            ohrep = persist.tile([128, NSLOT, E], f32, tag="ohrep")
            nc.sync.dma_start(
                ohrep[:],
                ohL.ap().rearrange("(o l) e -> o l e", o=1).to_broadcast([128, NSLOT, E]))
            lo_rep = persist.tile([128, NSLOT], f32, tag="lo_rep")
            nc.sync.dma_start(
                lo_rep[:],
                slotlo.ap().rearrange("(o l) -> o l", o=1).to_broadcast([128, NSLOT]))

            nc.gpsimd.collective_compute(
                "AllGather", Alu.bypass, replica_groups=RG,
                ins=[agin.ap().opt()], outs=[call.ap().opt()])

            # deferred init (Activation queue is idle while the router computes)
            for k in range(NSLOT):
                for g in range(NGRP):
                    nc.scalar.dma_start(idxbg[k][g][:],
                                        idxinit.ap()[0:profile[k] + 1, :])
            nc.sync.dma_start(c2aug[T:T + 1, :], zero_f32[:])
            for h in range(2):
                for k in range(T // 128):
                    nc.sync.dma_start(outp2[h][k * 128:(k + 1) * 128, :], zero_sb[:])
                nc.sync.dma_start(outp2[h][T:T + 1, :], zero_sb[0:1, :])

            # ====== phase 2: dispatch for the NSLOT local slots ======
            idx_sb = []
            s_col = []
            with (
                tc.tile_pool(name="dsb", bufs=3) as dsb,
                tc.tile_pool(name="dps", bufs=2, space="PSUM") as dps,
            ):
                cf = persist.tile([128, T // 128, E], f32, tag="cfall")
                nc.sync.dma_start(cf[:], call.ap().rearrange("(c p) e -> p c e", p=128))
                c2sb = persist.tile([128, T // 128, NSLOT], f32)
                xg_hold = [persist.tile([128, profile[k] // 128, H], bf16,
                                        tag=f"xgh{k}", name=f"xgh{k}")
                           for k in range(NSLOT)]
                for k in range(NSLOT):
                    idx_sb.append(persist.tile([128, profile[k] // 128], i32,
                                               tag=f"idx{k}", name=f"idx{k}"))
                    s_col.append(persist.tile([128, profile[k] // 128], f32,
                                              tag=f"scol{k}", name=f"scol{k}"))

                # pre-pass: per-slot combine weights ce -> c2sb, then one
                # c2aug store (needed by all scol gathers)
                for k in range(NSLOT):
                    msk = dsb.tile([128, T // 128, E], f32, tag="msk")
                    nc.vector.tensor_tensor(
                        msk[:], cf[:],
                        ohrep[:, k:k + 1, :].to_broadcast([128, T // 128, E]),
                        op=Alu.mult)
                    ce = dsb.tile([128, T // 128], f32, tag="ce")
                    nc.vector.reduce_sum(ce[:], msk[:], axis=mybir.AxisListType.X)
                    nc.vector.tensor_copy(c2sb[:, :, k], ce[:])
                nc.sync.dma_start(
                    c2aug.ap()[0:T, :].rearrange("(c p) l -> p c l", p=128),
                    c2sb[:])

                for k in range(NSLOT):
                    A = profile[k]
                    m = dsb.tile([128, T // 128], f32, tag="m")
                    nc.vector.tensor_scalar(m[:], c2sb[:, :, k], 0.0, None,
                                            op0=Alu.not_equal)

                    # exclusive cumsum over global token order
                    csp = dps.tile([16, 1], f32, tag="csp")
                    nc.tensor.matmul(csp[:], m[:], ones128_sb[:], start=True, stop=True)
                    cs_sb = dsb.tile([16, 1], f32, tag="cs_sb")
                    nc.any.tensor_copy(cs_sb[:], csp[:])
                    csrep = dsb.tile([16, 128], f32, tag="csrep")
                    nc.vector.tensor_copy(csrep[:], cs_sb[:].to_broadcast([16, 128]))
                    posp = dps.tile([128, T // 128], f32, tag="posp")
                    nc.tensor.matmul(posp[:], u128_sb[:], m[:], start=True, stop=False)
                    nc.tensor.matmul(posp[:], csrep[:], u16_sb[:], start=False, stop=True)

                    # gate to [lo, lo+A)
                    tpos = dsb.tile([128, T // 128], f32, tag="tpos")
                    nc.vector.tensor_scalar(tpos[:], posp[:], lo_rep[:, k:k + 1], None,
                                            op0=Alu.subtract)
                    g1 = dsb.tile([128, T // 128], f32, tag="g1")
                    nc.vector.tensor_scalar(g1[:], tpos[:], 0.0, None, op0=Alu.is_ge)
                    g2 = dsb.tile([128, T // 128], f32, tag="g2")
                    nc.vector.tensor_scalar(g2[:], tpos[:], float(A), None, op0=Alu.is_lt)
                    nc.vector.tensor_tensor(m[:], m[:], g1[:], op=Alu.mult)
                    nc.vector.tensor_tensor(m[:], m[:], g2[:], op=Alu.mult)

                    # offsets: O = A + m*(tpos - A)   (unselected -> dump slot A)
                    of = dsb.tile([128, T // 128], f32, tag="of")
                    nc.vector.tensor_scalar(of[:], tpos[:], float(A), None, op0=Alu.subtract)
                    nc.vector.tensor_tensor(of[:], of[:], m[:], op=Alu.mult)
                    nc.vector.tensor_scalar(of[:], of[:], float(A), None, op0=Alu.add)
                    oi = dsb.tile([128, T // 128], i32, tag="oi")
                    nc.vector.tensor_copy(oi[:], of[:])

                    # scatter token ids (single-column [128,1] offset APs; 4
                    # independent target tensors so the WAW chains run in
                    # parallel)
                    for g in range(NGRP):
                        for c in range(T // 128 // NGRP):
                            cc = g * (T // 128 // NGRP) + c
                            nc.gpsimd.indirect_dma_start(
                                out=idxbg[k][g].ap(),
                                out_offset=bass.IndirectOffsetOnAxis(
                                    ap=oi[:, cc:cc + 1], axis=0),
                                in_=tokid_sb[:, cc:cc + 1], in_offset=None,
                                bounds_check=A, oob_is_err=False)

                    # merge the 4 scatter groups (min; DUMP is the identity)
                    # and this slot's gathers, all slot-local and all on the
                    # Pool/DVE queues so later slots never block earlier ones
                    NCK = A // 128
                    tg = []
                    for g in range(NGRP):
                        t = dsb.tile([128, CMAX // 128], i32, tag=f"tg{g}")
                        nc.gpsimd.dma_start(
                            t[:, 0:NCK],
                            idxbg[k][g].ap()[0:A, :].rearrange("(c p) o -> p (c o)", p=128))
                        tg.append(t)
                    # tree min-reduce of the NGRP scatter groups
                    stride = 1
                    while stride < NGRP:
                        for g in range(0, NGRP, 2 * stride):
                            nc.vector.tensor_tensor(
                                tg[g][:, 0:NCK], tg[g][:, 0:NCK],
                                tg[g + stride][:, 0:NCK], op=Alu.min)
                        stride *= 2
                    nc.vector.tensor_copy(idx_sb[k][:], tg[0][:, 0:NCK])
                    for ck in range(NCK):
                        nc.gpsimd.indirect_dma_start(
                            out=xg_hold[k][:, ck, :], out_offset=None,
                            in_=xfull16.ap(),
                            in_offset=bass.IndirectOffsetOnAxis(
                                ap=idx_sb[k][:, ck:ck + 1], axis=0),
                            bounds_check=T - 1, oob_is_err=False)
                    for ck in range(NCK):
                        nc.gpsimd.indirect_dma_start(
                            out=s_col[k][:, ck:ck + 1], out_offset=None,
                            in_=c2aug.ap(),
                            in_offset=bass.IndirectOffsetOnAxis(
                                ap=idx_sb[k][:, ck:ck + 1], axis=0),
                            element_offset=k,
                            bounds_check=T, oob_is_err=True)

            # ====== phase 3: expert MLP per slot ======
            with (
                tc.tile_pool(name="w1", bufs=3) as w1p,
                tc.tile_pool(name="w2", bufs=1) as w2p,
                tc.tile_pool(name="hbuf", bufs=1) as hbp,
                tc.tile_pool(name="xt", bufs=2) as xtp,
                tc.tile_pool(name="ysb", bufs=2) as ysp,
                tc.tile_pool(name="bias", bufs=1) as biasp,
                tc.tile_pool(name="psh", bufs=3, space="PSUM") as psh,
                tc.tile_pool(name="psy", bufs=4, space="PSUM") as psy,
                tc.tile_pool(name="pst", bufs=1, space="PSUM") as pst,
            ):
                b1_sb = biasp.tile([128, NSLOT, NFC], f32)
                nc.sync.dma_start(b1_sb[:], b1L.ap().rearrange("l (c p) -> p l c", p=128))

                for k in range(NSLOT):
                    A = profile[k]
                    NCK = A // 128
                    # transpose gathered x rows on the PE (the HW serializes
                    # xbar DMA-transposes against other SBUF DMAs, which
                    # starves the weight stream — measured slower than PE)
                    xt = xtp.tile([128, H // 128, CMAX], bf16, tag="xt")
                    for ck in range(NCK):
                        for hc in range(H // 128):
                            tp = pst.tile([128, 128], bf16, tag="tp3")
                            nc.tensor.transpose(
                                tp[:], xg_hold[k][:, ck, hc * 128:(hc + 1) * 128],
                                ident16[:])
                            nc.any.tensor_copy(
                                xt[:, hc, ck * 128:(ck + 1) * 128], tp[:])

                    b2_sb = biasp.tile([1, H], bf16, tag="b2_sb")
                    nc.sync.dma_start(b2_sb[:], b2L.ap()[k:k + 1, :])

                    # mm1: h^T[f, c] = gelu(sum_h W1[h,f]^T x^T[h,c] + b1[f])
                    chs = [A] if A <= 512 else ([A // 2, A // 2] if A <= 1024
                                                else [512, 512, A - 1024])
                    hbuf = hbp.tile([128, NFC, CMAX], bf16, tag="hbuf")
                    for fo in range(NFG):
                        w1f = w1p.tile([128, 8, 512], bf16, tag="w1f")
                        nc.sync.dma_start(
                            w1f[:], W1L[k, fo].rearrange("p (c f) -> p c f", c=8))
                        for fi in range(4):
                            fg = fo * 4 + fi
                            cc0 = 0
                            for ch in chs:
                                ph = psh.tile([128, 512], f32, tag="ph")
                                for hc in range(H // 128):
                                    nc.tensor.matmul(
                                        ph[:, 0:ch],
                                        w1f[:, hc, fi * 128:(fi + 1) * 128],
                                        xt[:, hc, cc0:cc0 + ch],
                                        start=(hc == 0), stop=(hc == H // 128 - 1))
                                nc.scalar.activation(
                                    hbuf[:, fg, cc0:cc0 + ch], ph[:, 0:ch],
                                    Act.Gelu, bias=b1_sb[:, k, fg:fg + 1])
                                cc0 += ch

                    # W2 slice resident for the slot; the DMA overlaps the
                    # mm1 tail (emitted after mm1's w1f loads on the SP queue)
                    w2r = w2p.tile([128, NFC, H], bf16, tag="w2r")
                    nc.sync.dma_start(w2r[:], W2L[k])

                    # mm2: y[c, h] = (sum_f h^T[f,c]^T W2[f,h] + b2[h]) * s[c]
                    for hh in range(2):
                        for tb0 in range(0, NCK, 4):
                            tbn = min(4, NCK - tb0)
                            pys = [psy.tile([128, HH], f32, tag="py", name=f"py{_i}")
                                   for _i in range(tbn)]
                            for fc in range(NFC):
                                for i in range(tbn):
                                    ck = tb0 + i
                                    nc.tensor.matmul(
                                        pys[i][:],
                                        hbuf[:, fc, ck * 128:(ck + 1) * 128],
                                        w2r[:, fc, hh * HH:(hh + 1) * HH],
                                        start=(fc == 0), stop=False)
                            ysb = ysp.tile([128, 4, HH], bf16, tag="ysb")
                            for i in range(tbn):
                                ck = tb0 + i
                                nc.tensor.matmul(
                                    pys[i][:], onesmm_sb[0:1, :],
                                    b2_sb[0:1, hh * HH:(hh + 1) * HH],
                                    start=False, stop=True)
                                nc.scalar.activation(
                                    ysb[:, i, :], pys[i][:], Act.Copy,
                                    scale=s_col[k][:, ck:ck + 1])
                                nc.gpsimd.indirect_dma_start(
                                    out=outp2[hh].ap(),
                                    out_offset=bass.IndirectOffsetOnAxis(
                                        ap=idx_sb[k][:, ck:ck + 1], axis=0),
                                    in_=ysb[:, i, :], in_offset=None,
                                    compute_op=Alu.add,
                                    bounds_check=T, oob_is_err=True)
                        if k == NSLOT - 1 and hh == 0:
                            # all h-half-0 contributions are in: overlap its
                            # ReduceScatter with h-half-1 compute
                            nc.gpsimd.collective_compute(
                                "ReduceScatter", Alu.add, replica_groups=RG,
                                ins=[outp2[0].ap()[0:T, :].opt()],
                                outs=[rsout2[0].ap().opt()])

            # ====== phase 4: remaining reduce + output shard ======
            with tc.tile_pool(name="outc", bufs=2) as outc:
                for k in range(TSH // 128):
                    ot = outc.tile([128, HH], bf16, tag="ot")
                    nc.sync.dma_start(ot[:], rsout2[0][k * 128:(k + 1) * 128, :])
                    otf = outc.tile([128, HH], f32, tag="otf")
                    nc.vector.tensor_copy(otf[:], ot[:])
                    nc.sync.dma_start(out_sh[k * 128:(k + 1) * 128, 0:HH], otf[:])
                nc.gpsimd.collective_compute(
                    "ReduceScatter", Alu.add, replica_groups=RG,
                    ins=[outp2[1].ap()[0:T, :].opt()], outs=[rsout2[1].ap().opt()])
                for k in range(TSH // 128):
                    ot = outc.tile([128, HH], bf16, tag="ot")
                    nc.sync.dma_start(ot[:], rsout2[1][k * 128:(k + 1) * 128, :])
                    otf = outc.tile([128, HH], f32, tag="otf")
                    nc.vector.tensor_copy(otf[:], ot[:])
                    nc.sync.dma_start(out_sh[k * 128:(k + 1) * 128, HH:H], otf[:])

    nc.compile()
    if not nc.is_finalized():
        nc.finalize()
    return nc


# ---------------------------------------------------------------------------
# Host-side input preparation
# ---------------------------------------------------------------------------

def _in_maps(inputs, profile, asg):
    import ml_dtypes
    bf16 = ml_dtypes.bfloat16

    NSLOT = len(profile)
    NFG = FH // 512
    NFC = FH // 128
    x = np.ascontiguousarray(np.asarray(inputs["x"], np.float32).reshape(T, H))
    W1 = np.asarray(inputs["W1"], np.float32)
    b1 = np.asarray(inputs["b1"], np.float32)
    W2 = np.asarray(inputs["W2"], np.float32)
    b2 = np.asarray(inputs["b2"], np.float32)
    common = {
        "xfull16": np.ascontiguousarray(x.astype(bf16)),
        "Wr1": np.ascontiguousarray(np.asarray(inputs["Wr1"], np.float32)),
        "br1": np.ascontiguousarray(np.asarray(inputs["br1"], np.float32)),
        "Wr2": np.ascontiguousarray(np.asarray(inputs["Wr2"], np.float32)),
        "br2": np.ascontiguousarray(np.asarray(inputs["br2"], np.float32)),
    }
    maps = []
    for r in range(NCORES):
        w1l = np.empty((NSLOT, NFG, 128, 8 * 512), bf16)
        w2l = np.empty((NSLOT, 128, NFC, H), bf16)
        b1l = np.zeros((NSLOT, FH), np.float32)
        b2l = np.zeros((NSLOT, H), bf16)
        oh = np.zeros((NSLOT, E), np.float32)
        lo = np.zeros((NSLOT,), np.float32)
        for k in range(NSLOT):
            (e, hf), l0 = asg[k][r]
            f0 = hf * FH
            w1h = W1[e][:, f0:f0 + FH]                    # [H, FH]
            # W1L[k, fo, p, hc*512+fc] = w1h[hc*128+p, fo*512+fc]
            w1l[k] = (w1h.reshape(8, 128, NFG, 512)       # hc, p, fo, fc
                      .transpose(2, 1, 0, 3)              # fo, p, hc, fc
                      .reshape(NFG, 128, 8 * 512).astype(bf16))
            w2h = W2[e][f0:f0 + FH, :]                    # [FH, H]
            # W2L[k, p, fc, ho] = w2h[fc*128+p, ho]
            w2l[k] = (w2h.reshape(NFC, 128, H)
                      .transpose(1, 0, 2).astype(bf16))
            b1l[k] = b1[e][f0:f0 + FH]
            if hf == 0:
                b2l[k] = b2[e].astype(bf16)
            if l0 <= T:
                oh[k, e] = 1.0       # empty slots keep an all-zero one-hot
            lo[k] = float(l0)
        # shard x^T for the router: xshT[p, hc, t] = x[r*TSH+t, hc*128+p]
        xs = x[r * TSH:(r + 1) * TSH]                     # [TSH, H]
        xshT = np.ascontiguousarray(
            xs.T.reshape(8, 128, TSH).transpose(1, 0, 2))
        maps.append({
            **common,
            "xshT": xshT,
            "W1L": w1l, "W2L": w2l, "b1L": b1l, "b2L": b2l,
            "ohL": oh, "slotlo": lo,
        })
    return maps


def _get_nc(profile):
    key = tuple(profile)
    if key not in _CACHE:
        _CACHE[key] = _build(list(key))
    return _CACHE[key]


def kernel(**inputs) -> np.ndarray:
    from concourse.bass_utils import run_bass_kernel_spmd

    profile, asg = _plan(inputs)
    nc = _get_nc(profile)
    maps = _in_maps(inputs, profile, asg)
    res = run_bass_kernel_spmd(nc, maps, core_ids=list(range(NCORES)))
    shards = [res.results[r]["out_sh"] for r in range(NCORES)]
    out = np.concatenate(shards, axis=0).reshape(np.asarray(inputs["x"]).shape)
    return out.astype(np.float32)
